# revision 35
# baseline (speedup 1.0000x reference)
"""Trainium2 Bass kernel for single-CLS-query attention.

Reference computation (per batch b):
    q   = (x[b,0,:] @ Wq.T) * d**-0.5                  # (C,)  single CLS query
    k   = x[b] @ Wk.T ; v = x[b] @ Wv.T                # (N,C)
    s   = per-head dot(q, k) + mask                    # (N,H)
    p   = softmax(s, axis=N)
    out = per-head sum_n p[n,h] v[n,h*64:(h+1)*64]     # (C,)
    y   = out @ Wp.T + bp

Key algebraic restructuring (exploits the single query):
    qhat[h,:] = sum_d q[h*64+d] * Wk[h*64+d,:]         # (H,C)  fold q through Wk
    s         = x @ qhat.T                             # skinny matmul, no k!
    z[h,:]    = sum_n p[n,h] * x[b,n,:]                # (H,C)  fold p into x
    out'      = z @ Wv.T  (full 16x1024 cross)         # block-diag extract -> out
This removes both dense projections x@Wk.T / x@Wv.T (~137 GFLOP -> ~2 GFLOP)
and makes the kernel memory-bound on streaming x.

Implementation notes (final):
  * s in NATURAL orientation: per 128-row n-tile,
    s_nat(128n x 16h) = sum_k xt_tile(128c x 128n).T @ qhatT(128c x 16h);
    the transposed-x tile is the PE stationary operand (FWL LDWEIGHTS), the
    tiny qhatT is moving. No on-chip transposes of x; softmax reads logits
    straight from PSUM.
  * BOTH x copies ship as fp8e4m3 (qhat/p/weights stay bf16): the logit and
    z quantization noise averages over the 4096-wide softmax reduction;
    measured end-to-end rel err 1.68e-2 vs the 2e-2 gate (bf16 x gives
    3.6e-3 at ~20us slower, xt-only-fp8 gives 9.2e-3 at ~8us slower).
  * mask rides the Act exp for free: p = Exp(s + mask_bias), mask host-packed
    as a (128,1) per-partition bias column per n-tile.
  * ~25 DMAs total, every bulk tensor host-packed per-partition-contiguous
    (128 descriptors of 8-16KB each => sub-us HWDGE issue), all emitted
    upfront on one ring so pool-buffer semaphores throttle issue into an
    automatic ~5-quarter read-ahead, decoupled from compute progress.
  * software pipelining: each n-tile's z-matmuls are emitted two tiles late
    so the Act exp hides under the next s-chains; per-tile PSUM/SBUF tiles
    with small pool depths force the scheduler to interleave s- and z-work
    finely, which keeps the PE HAM clock gate at 2.4GHz (quarter-batched
    schedules oscillate 1.2/2.4GHz).
  * z-finalize (1/l scale + transpose) is sliced into thunks interleaved one
    per n-tile into the next batch's stream; the two batches' out'/y
    projections are merged into ONE weight pass each, computed TRANSPOSED
    (stationary Wv/Wp 128x128 slices on the clock-immune LDWEIGHTS path) so
    the block-diag extract is two strided DVE copies per batch and y lands
    c-major for a contiguous store.
  * dummy-matmul burst at start warms the PE HAM clock gate during DMA fill.

Sharding: data-parallel over batch. 8 cores x 2 batches each. No collectives.
softmax runs without max-subtraction: logits are ~N(0,0.4), far inside fp32
exp range.
"""

import numpy as np
from contextlib import ExitStack

import concourse.bass as bass
from concourse import bacc
import concourse.tile as tile
from concourse import mybir
from concourse import bass_utils
from concourse.masks import make_identity

B, N, C, H, D = 16, 4096, 1024, 16, 64
NCORES = 8
BPC = B // NCORES          # batches per core
SCALE = float(D) ** -0.5
F32 = mybir.dt.float32
BF16 = mybir.dt.bfloat16
FP8 = mybir.dt.float8e4
NT = N // 128              # 32 n-tiles of 128 rows
CB = C // 128              # 8 column blocks
NQ = 4                     # quarters per batch (8 n-tiles each)
TPQ = NT // NQ             # n-tiles per quarter
NPQ = N // NQ              # rows per quarter

XT_DT = FP8                # dtype of the transposed-x copy (s path)
XIN_DT = FP8               # dtype of the natural-x copy (z path)

AF = mybir.ActivationFunctionType
ALU = mybir.AluOpType


def build_module():
    nc = bacc.Bacc(target_bir_lowering=False, trn_type="TRN2")

    # all bulk tensors are host-packed so each DMA is per-partition contiguous
    x_d = nc.dram_tensor("xb", [BPC, NQ, 128, TPQ * C], XIN_DT, kind="ExternalInput")
    xt_d = nc.dram_tensor("xtb", [BPC, NQ, 128, CB * NPQ], XT_DT, kind="ExternalInput")
    qh_d = nc.dram_tensor("qhp", [128, BPC * CB * H], BF16, kind="ExternalInput")
    mk_d = nc.dram_tensor("mkp", [128, BPC * NT], F32, kind="ExternalInput")
    wvt_d = nc.dram_tensor("WvT", [128, CB * C], BF16, kind="ExternalInput")
    wpt_d = nc.dram_tensor("WpT", [128, CB * C], BF16, kind="ExternalInput")
    bp_d = nc.dram_tensor("bp", [128, CB], F32, kind="ExternalInput")
    y_d = nc.dram_tensor("y", [BPC, C], F32, kind="ExternalOutput")

    with tile.TileContext(nc) as tc, ExitStack() as ctx:
        singles = ctx.enter_context(tc.tile_pool(name="singles", bufs=1))
        xtp = ctx.enter_context(tc.tile_pool(name="xtp", bufs=5))
        xip = ctx.enter_context(tc.tile_pool(name="xip", bufs=5))
        pp = ctx.enter_context(tc.tile_pool(name="pp", bufs=4))
        psS = ctx.enter_context(tc.tile_pool(name="psS", bufs=2, space="PSUM"))
        psAcc = ctx.enter_context(tc.tile_pool(name="psAcc", bufs=2, space="PSUM"))
        psL = ctx.enter_context(tc.tile_pool(name="psL", bufs=1, space="PSUM"))
        psT = ctx.enter_context(tc.tile_pool(name="psT", bufs=1, space="PSUM"))

        ident = singles.tile([128, 128], F32)
        make_identity(nc, ident)

        ones_col = singles.tile([128, 1], BF16)
        nc.vector.memset(ones_col, 1.0)

        # ---- PE warm-up: matmul burst sized to span the whole DMA fill
        # window (~10us: sequencer init ~7us + first x quarter ~3us), so the
        # HAM clock gate is open AND stays open when the first z-matmuls
        # arrive -- a short burst decays during the fill gap and the stream
        # then starts at 1.2GHz.
        wsrc = singles.tile([128, 512], BF16)
        nc.vector.memset(wsrc, 0.0)
        warm_ps = psT.tile([1, 512], F32, tag="tp", name="warm_ps")
        for i in range(34):
            nc.tensor.matmul(warm_ps, ones_col, wsrc, start=(i == 0), stop=(i == 33))

        # ---- streamed x tiles: quarters of a batch (per-partition contiguous)
        xt_tiles = {}
        xin_tiles = {}

        def emit_quarter(b, q, eng=None):
            if b >= BPC:
                return
            eng = eng or nc.sync
            xt = xtp.tile([128, CB, NPQ], XT_DT, tag="xt")
            xin = xip.tile([128, TPQ, C], XIN_DT, tag="xin")
            xt_src = xt_d[b, q].rearrange("p (k n) -> p k n", k=CB)
            xin_src = x_d[b, q].rearrange("p (t c) -> p t c", t=TPQ)
            eng.dma_start(out=xt, in_=xt_src)
            eng.dma_start(out=xin, in_=xin_src)
            xt_tiles[(b, q)] = xt
            xin_tiles[(b, q)] = xin

        # ---- small inputs first (tiny drains, needed by the first n-tile) ----
        qhT = singles.tile([128, BPC, CB, H], BF16)
        nc.sync.dma_start(out=qhT, in_=qh_d.rearrange("p (b k h) -> p b k h", b=BPC, k=CB))
        mkT = singles.tile([128, BPC, NT], F32)
        nc.sync.dma_start(out=mkT, in_=mk_d.rearrange("p (b t) -> p b t", b=BPC))
        bpT = singles.tile([128, CB], F32)
        nc.sync.dma_start(out=bpT, in_=bp_d[:, :])

        # ---- the whole bulk-DMA program, emitted upfront: batch-0 quarters,
        # then weights interleaved ahead of the batch-1 quarters. Issue of
        # each transfer waits only on its pool buffer being free, so the DMA
        # engines read ahead as far as SBUF allows, decoupled from compute.
        wvT = singles.tile([128, CB, C], BF16)
        wpT = singles.tile([128, CB, C], BF16)
        for q in range(NQ):
            emit_quarter(0, q)
        nc.sync.dma_start(out=wvT, in_=wvt_d.rearrange("p (k c) -> p k c", k=CB))
        emit_quarter(1, 0)
        nc.sync.dma_start(out=wpT, in_=wpt_d.rearrange("p (k c) -> p k c", k=CB))
        for q in range(1, NQ):
            emit_quarter(1, q)

        ocb = singles.tile([128, BPC, CB], BF16)       # extracted out columns
        zTb = singles.tile([128, CB, BPC * H], BF16)   # packed z.T, both batches

        def emit_z(z_ps, l_ps, p_nat, tt, xin, b, t):
            # z += p.T @ x ; l += p.T @ ones  (whole-batch accumulation);
            # returned as three thunks the caller interleaves into the s-chain
            first, last = (t == 0), (t == NT - 1)
            def zcc(cc):
                return lambda: nc.tensor.matmul(
                    z_ps[:, cc * 512:(cc + 1) * 512],
                    p_nat[:, :],
                    xin[:, tt, cc * 512:(cc + 1) * 512],
                    start=first,
                    stop=last,
                )
            return (zcc(0), zcc(1), lambda: nc.tensor.matmul(
                l_ps, p_nat[:, :], ones_col, start=first, stop=last))

        zq = []
        epi_thunks = []
        for b in range(BPC):
            z_ps = psAcc.tile([H, C], F32, tag="acc")
            l_ps = psL.tile([H, 1], F32, tag="l")

            for t in range(NT):
                q, tt = divmod(t, TPQ)
                s_ps = psS.tile([128, H], F32, tag="s")
                p_nat = pp.tile([128, H], BF16, tag="p")
                xt = xt_tiles[(b, q)]
                xin = xin_tiles[(b, q)]

                # s_nat(128n, 16h) = sum_k xt_tile.T @ qhatT  (xt stationary, FWL)
                for k in range(CB):
                    nc.tensor.matmul(
                        s_ps,
                        xt[:, k, tt * 128:(tt + 1) * 128],
                        qhT[:, b, k, :],
                        start=(k == 0),
                        stop=(k == CB - 1),
                    )
                for f in (list(emit_z(*zq.pop(0))) if len(zq) >= 2 else []):
                    f()
                # p = exp(s + mask) with per-partition mask bias, straight from PSUM
                nc.scalar.activation(
                    out=p_nat,
                    in_=s_ps,
                    func=AF.Exp,
                    bias=mkT[:, b, t:t + 1],
                )
                zq.append((z_ps, l_ps, p_nat, tt, xin, b, t))
                if epi_thunks:
                    epi_thunks.pop(0)()

            # flush the pipelined z-chains for this batch's last n-tiles
            while zq:
                for f in emit_z(*zq.pop(0)):
                    f()

            # ---- epilogue thunks: z-finalize for this batch (interleaved
            # into the next batch's n-tile stream, one thunk per tile) ----
            def make_epilogue(b, z_ps, l_ps):
                th = []
                if b == 0:
                    # small matmul burst, slotted into the batch boundary by
                    # pool rotation (its buffer waits l_ps's reciprocal read):
                    # keeps the HAM clock gate open across the z-finalize dip
                    # so batch 1's first z-matmuls run at full clock
                    wb = psL.tile([1, 512], F32, tag="l", name="wb0")

                    def warmburst():
                        for i in range(3):
                            nc.tensor.matmul(wb, ones_col, wsrc,
                                             start=(i == 0), stop=(i == 2))
                    th.append(warmburst)
                linv = singles.tile([H, 1], F32, name=f"linv{b}")
                z_sb = singles.tile([H, C], F32, name=f"z_sb{b}")
                ztp = psT.tile([128, CB, H], F32, tag="tp", name=f"ztp{b}")

                th.append(lambda: nc.vector.reciprocal(out=linv, in_=l_ps))
                for hh in range(2):
                    th.append(lambda hh=hh: nc.vector.tensor_scalar_mul(
                        z_sb[:, hh * 512:(hh + 1) * 512],
                        z_ps[:, hh * 512:(hh + 1) * 512], linv))
                for k0 in range(0, CB, 2):
                    def tr(k0=k0):
                        for k in (k0, k0 + 1):
                            nc.tensor.transpose(
                                ztp[:, k, :], z_sb[:, k * 128:(k + 1) * 128],
                                ident[0:H, 0:H])
                    th.append(tr)
                th.append(lambda: nc.vector.tensor_copy(
                    out=zTb[:, :, b * H:(b + 1) * H],
                    in_=ztp.rearrange("p k h -> p k h")))
                return th

            epi_thunks.extend(make_epilogue(b, z_ps, l_ps))

        for th in epi_thunks:
            th()

        # ---- merged projections, both batches in one weight pass ----
        # out'T[c', (b,h)] = (z @ Wv.T).T via stationary Wv slices: output is
        # c-major so the block-diag extract is two strided DVE copies per batch
        OP = psT.tile([128, CB, BPC * H], F32, tag="tp", name="OP")
        for m in range(CB):
            for k in range(CB):
                nc.tensor.matmul(
                    OP[:, m, :],
                    wvT[:, k, m * 128:(m + 1) * 128],
                    zTb[:, k, :],
                    start=(k == 0),
                    stop=(k == CB - 1),
                )
        # ocb[p, b, j] = OP[p, j, b*H + 2j + (p >= 64)]
        for b in range(BPC):
            ev = OP[0:64, 0, b * H:b * H + 1]
            od = OP[64:128, 0, b * H + 1:b * H + 2]
            nc.vector.tensor_copy(
                out=ocb[0:64, b, :],
                in_=bass.AP(tensor=ev.tensor, offset=ev.offset,
                            ap=[ev.ap[0], [BPC * H + 2, CB]]))
            nc.vector.tensor_copy(
                out=ocb[64:128, b, :],
                in_=bass.AP(tensor=od.tensor, offset=od.offset,
                            ap=[od.ap[0], [BPC * H + 2, CB]]))

        # yT[c2, b] = (out @ Wp.T).T via stationary Wp slices
        YT = psL.tile([128, CB, BPC], F32, tag="l", name="YT")
        for m in range(CB):
            for j in range(CB):
                nc.tensor.matmul(
                    YT[:, m, :],
                    wpT[:, j, m * 128:(m + 1) * 128],
                    ocb[:, :, j],
                    start=(j == 0),
                    stop=(j == CB - 1),
                )
        y_sb = singles.tile([128, CB, BPC], F32)
        for b in range(BPC):
            nc.vector.tensor_tensor(
                out=y_sb[:, :, b], in0=YT[:, :, b], in1=bpT, op=ALU.add)
            nc.sync.dma_start(
                out=y_d[b, :].rearrange("(m p) -> p m", p=128), in_=y_sb[:, :, b]
            )

    nc.compile()
    return nc


def _ensure_ntff_hook():
    """The agent image's antenv lacks axon_hooks; synthesize it and install
    the ctypes NTFF profile hook from trn_boot so trace=True works."""
    import sys
    import types
    try:
        from antenv.axon_hooks import get_axon_ntff_profile_hook  # noqa: F401
        return
    except ImportError:
        pass
    import antenv
    mod = types.ModuleType("antenv.axon_hooks")
    state = {}
    mod.set_axon_ntff_profile_hook = lambda h: state.__setitem__("h", h)
    mod.get_axon_ntff_profile_hook = lambda: state.get("h")
    sys.modules["antenv.axon_hooks"] = mod
    antenv.axon_hooks = mod
    try:
        from trn_agent_boot.trn_boot import _ntff_profile_via_ctypes
        mod.set_axon_ntff_profile_hook(
            _ntff_profile_via_ctypes("/opt/axon/libaxon_pjrt.so")
        )
    except Exception:
        pass


_NC_CACHE = None


def _get_module():
    global _NC_CACHE
    if _NC_CACHE is None:
        _NC_CACHE = build_module()
    return _NC_CACHE


def _np_xt_dtype():
    import ml_dtypes
    return {BF16: ml_dtypes.bfloat16, FP8: ml_dtypes.float8_e4m3fn}[XT_DT]


def _prep_inputs(inputs):
    """Host-side prep: bf16/fp8 casts and per-partition-contiguous packing."""
    import ml_dtypes
    bf16 = ml_dtypes.bfloat16

    x = np.ascontiguousarray(inputs["x"], dtype=np.float32)       # (B,N,C)
    mask = np.ascontiguousarray(inputs["mask"], dtype=np.float32)
    Wq = np.asarray(inputs["Wq"], dtype=np.float32)
    Wk = np.asarray(inputs["Wk"], dtype=np.float32)

    # natural x, packed [b, q, p, (t c)]: partition p = n%128 within quarter
    xb = np.ascontiguousarray(
        x.reshape(B, NQ, TPQ, 128, C).transpose(0, 1, 3, 2, 4)
    ).reshape(B, NQ, 128, TPQ * C).astype(_np_xt_dtype())
    # transposed x, packed [b, q, p, (k n)]: partition p = c%128
    xtb = np.ascontiguousarray(
        x.transpose(0, 2, 1).reshape(B, CB, 128, NQ, NPQ).transpose(0, 3, 2, 1, 4)
    ).reshape(B, NQ, 128, CB * NPQ).astype(_np_xt_dtype())

    # qhat[b,h,:] = sum_d (x[b,0] @ Wq.T * scale)[h*64+d] * Wk[h*64+d,:]
    q = (x[:, 0, :].astype(np.float64) @ Wq.T.astype(np.float64)) * SCALE  # (B,C)
    qhd = q.reshape(B, H, D)
    Wkh = Wk.reshape(H, D, C).astype(np.float64)
    qhat = np.einsum("bhd,hdc->bhc", qhd, Wkh)                     # (B,H,C)
    qhT = qhat.transpose(0, 2, 1)                                  # (B,C,H)
    qhp = np.ascontiguousarray(
        qhT.reshape(NCORES, BPC, CB, 128, H).transpose(0, 3, 1, 2, 4)
    ).reshape(NCORES, 128, BPC * CB * H).astype(bf16)

    # mask_full packed per n-tile: (core, 128, BPC*NT)
    mask_full = np.concatenate(
        [np.zeros((B, 1), dtype=np.float32), mask], axis=1)        # (B,N)
    mkp = np.ascontiguousarray(
        mask_full.reshape(NCORES, BPC, NT, 128).transpose(0, 3, 1, 2)
    ).reshape(NCORES, 128, BPC * NT)

    def packw(w):
        wt = np.ascontiguousarray(np.asarray(w, dtype=np.float32).T)  # (C,C)
        return np.ascontiguousarray(
            wt.reshape(CB, 128, C).transpose(1, 0, 2)
        ).reshape(128, CB * C).astype(bf16)

    shared = {
        "WvT": packw(inputs["Wv"]),
        "WpT": packw(inputs["Wp"]),
        "bp": np.ascontiguousarray(
            np.asarray(inputs["bp"], dtype=np.float32).reshape(CB, 128).T),
    }
    in_maps = []
    for c in range(NCORES):
        sl = slice(c * BPC, (c + 1) * BPC)
        m = {
            "xb": xb[sl], "xtb": xtb[sl], "qhp": qhp[c], "mkp": mkp[c],
        }
        m.update(shared)
        in_maps.append(m)
    return in_maps


def run(inputs, trace=False):
    if trace:
        _ensure_ntff_hook()
    nc = _get_module()
    in_maps = _prep_inputs(inputs)
    res = bass_utils.run_bass_kernel_spmd(
        nc, in_maps, core_ids=list(range(NCORES)), trace=trace
    )
    ys = [res.results[c]["y"] for c in range(NCORES)]
    out = np.concatenate(ys, axis=0).reshape(B, 1, C)
    return out, res


def kernel(**inputs):
    out, _ = run(inputs, trace=False)
    return out


if __name__ == "__main__":
    rng = np.random.default_rng(0)
    ins = {
        "x": rng.standard_normal((B, N, C), dtype=np.float32),
        "mask": np.zeros((B, N - 1), dtype=np.float32),
        "Wq": (rng.standard_normal((C, C)) * 0.02).astype(np.float32),
        "Wk": (rng.standard_normal((C, C)) * 0.02).astype(np.float32),
        "Wv": (rng.standard_normal((C, C)) * 0.02).astype(np.float32),
        "Wp": (rng.standard_normal((C, C)) * 0.02).astype(np.float32),
        "bp": np.zeros((C,), dtype=np.float32),
    }
    y = kernel(**ins)
    print(y.shape, y.dtype, np.abs(y).mean())


# revision 37
# speedup vs baseline: 1.0302x; 1.0302x over previous
"""Trainium2 Bass kernel for single-CLS-query attention.

Reference computation (per batch b):
    q   = (x[b,0,:] @ Wq.T) * d**-0.5                  # (C,)  single CLS query
    k   = x[b] @ Wk.T ; v = x[b] @ Wv.T                # (N,C)
    s   = per-head dot(q, k) + mask                    # (N,H)
    p   = softmax(s, axis=N)
    out = per-head sum_n p[n,h] v[n,h*64:(h+1)*64]     # (C,)
    y   = out @ Wp.T + bp

Key algebraic restructuring (exploits the single query):
    qhat[h,:] = sum_d q[h*64+d] * Wk[h*64+d,:]         # (H,C)  fold q through Wk
    s         = x @ qhat.T                             # skinny matmul, no k!
    z[h,:]    = sum_n p[n,h] * x[b,n,:]                # (H,C)  fold p into x
    out'      = z @ Wv.T  (full 16x1024 cross)         # block-diag extract -> out
This removes both dense projections x@Wk.T / x@Wv.T (~137 GFLOP -> ~2 GFLOP)
and makes the kernel memory-bound on streaming x.

Implementation notes (final):
  * s in NATURAL orientation: per 128-row n-tile,
    s_nat(128n x 16h) = sum_k xt_tile(128c x 128n).T @ qhatT(128c x 16h);
    the transposed-x tile is the PE stationary operand (FWL LDWEIGHTS), the
    tiny qhatT is moving. No on-chip transposes of x; softmax reads logits
    straight from PSUM.
  * BOTH x copies ship as fp8e4m3 (qhat/p/weights stay bf16): the logit and
    z quantization noise averages over the 4096-wide softmax reduction;
    measured end-to-end rel err 1.68e-2 vs the 2e-2 gate (bf16 x gives
    3.6e-3 at ~20us slower, xt-only-fp8 gives 9.2e-3 at ~8us slower).
  * mask rides the Act exp for free: p = Exp(s + mask_bias), mask host-packed
    as a (128,1) per-partition bias column per n-tile.
  * ~25 DMAs total, every bulk tensor host-packed per-partition-contiguous
    (128 descriptors of 8-16KB each => sub-us HWDGE issue), all emitted
    upfront on one ring so pool-buffer semaphores throttle issue into an
    automatic ~5-quarter read-ahead, decoupled from compute progress.
  * software pipelining: each n-tile's z-matmuls are emitted two tiles late
    so the Act exp hides under the next s-chains; per-tile PSUM/SBUF tiles
    with small pool depths force the scheduler to interleave s- and z-work
    finely, which keeps the PE HAM clock gate at 2.4GHz (quarter-batched
    schedules oscillate 1.2/2.4GHz).
  * z-finalize (1/l scale + transpose) is sliced into thunks interleaved one
    per n-tile into the next batch's stream; the two batches' out'/y
    projections are merged into ONE weight pass each, computed TRANSPOSED
    (stationary Wv/Wp 128x128 slices on the clock-immune LDWEIGHTS path) so
    the block-diag extract is two strided DVE copies per batch and y lands
    c-major for a contiguous store.
  * dummy-matmul burst at start warms the PE HAM clock gate during DMA fill.

Sharding: data-parallel over batch. 8 cores x 2 batches each. No collectives.
softmax runs without max-subtraction: logits are ~N(0,0.4), far inside fp32
exp range.
"""

import numpy as np
from contextlib import ExitStack

import concourse.bass as bass
from concourse import bacc
import concourse.tile as tile
from concourse import mybir
from concourse import bass_utils
from concourse.masks import make_identity

B, N, C, H, D = 16, 4096, 1024, 16, 64
NCORES = 8
BPC = B // NCORES          # batches per core
SCALE = float(D) ** -0.5
F32 = mybir.dt.float32
BF16 = mybir.dt.bfloat16
FP8 = mybir.dt.float8e4
NT = N // 128              # 32 n-tiles of 128 rows
CB = C // 128              # 8 column blocks
NQ = 4                     # quarters per batch (8 n-tiles each)
TPQ = NT // NQ             # n-tiles per quarter
NPQ = N // NQ              # rows per quarter

XT_DT = FP8                # dtype of the transposed-x copy (s path)
XIN_DT = FP8               # dtype of the natural-x copy (z path)

AF = mybir.ActivationFunctionType
ALU = mybir.AluOpType


def build_module():
    nc = bacc.Bacc(target_bir_lowering=False, trn_type="TRN2")

    # all bulk tensors are host-packed so each DMA is per-partition contiguous
    x_d = nc.dram_tensor("xb", [BPC, NQ, 128, TPQ * C], XIN_DT, kind="ExternalInput")
    xt_d = nc.dram_tensor("xtb", [BPC, NQ, 128, CB * NPQ], XT_DT, kind="ExternalInput")
    qh_d = nc.dram_tensor("qhp", [128, BPC * CB * H], BF16, kind="ExternalInput")
    mk_d = nc.dram_tensor("mkp", [128, BPC * NT], F32, kind="ExternalInput")
    wvt_d = nc.dram_tensor("WvT", [128, CB * C], BF16, kind="ExternalInput")
    wpt_d = nc.dram_tensor("WpT", [128, CB * C], BF16, kind="ExternalInput")
    bp_d = nc.dram_tensor("bp", [128, CB], F32, kind="ExternalInput")
    y_d = nc.dram_tensor("y", [BPC, C], F32, kind="ExternalOutput")

    with tile.TileContext(nc) as tc, ExitStack() as ctx:
        singles = ctx.enter_context(tc.tile_pool(name="singles", bufs=1))
        xtp = ctx.enter_context(tc.tile_pool(name="xtp", bufs=5))
        xip = ctx.enter_context(tc.tile_pool(name="xip", bufs=5))
        pp = ctx.enter_context(tc.tile_pool(name="pp", bufs=4))
        psS = ctx.enter_context(tc.tile_pool(name="psS", bufs=2, space="PSUM"))
        psAcc = ctx.enter_context(tc.tile_pool(name="psAcc", bufs=2, space="PSUM"))
        psL = ctx.enter_context(tc.tile_pool(name="psL", bufs=1, space="PSUM"))
        psT = ctx.enter_context(tc.tile_pool(name="psT", bufs=1, space="PSUM"))

        ident = singles.tile([128, 128], F32)
        make_identity(nc, ident)

        ones_col = singles.tile([128, 1], BF16)
        nc.vector.memset(ones_col, 1.0)

        # ---- PE warm-up: matmul burst sized to span the whole DMA fill
        # window (~10us: sequencer init ~7us + first x quarter ~3us), so the
        # HAM clock gate is open AND stays open when the first z-matmuls
        # arrive -- a short burst decays during the fill gap and the stream
        # then starts at 1.2GHz.
        wsrc = singles.tile([128, 512], BF16)
        nc.vector.memset(wsrc, 0.0)
        warm_ps = psT.tile([1, 512], F32, tag="tp", name="warm_ps")
        for i in range(34):
            nc.tensor.matmul(warm_ps, ones_col, wsrc, start=(i == 0), stop=(i == 33))

        # ---- streamed x tiles: quarters of a batch (per-partition contiguous)
        xt_tiles = {}
        xin_tiles = {}

        def emit_quarter(b, q, eng=None):
            if b >= BPC:
                return
            eng = eng or nc.sync
            xt = xtp.tile([128, CB, NPQ], XT_DT, tag="xt")
            xin = xip.tile([128, TPQ, C], XIN_DT, tag="xin")
            xt_src = xt_d[b, q].rearrange("p (k n) -> p k n", k=CB)
            xin_src = x_d[b, q].rearrange("p (t c) -> p t c", t=TPQ)
            eng.dma_start(out=xt, in_=xt_src)
            eng.dma_start(out=xin, in_=xin_src)
            xt_tiles[(b, q)] = xt
            xin_tiles[(b, q)] = xin

        # ---- small inputs first (tiny drains, needed by the first n-tile) ----
        qhT = singles.tile([128, BPC, CB, H], BF16)
        nc.sync.dma_start(out=qhT, in_=qh_d.rearrange("p (b k h) -> p b k h", b=BPC, k=CB))
        mkT = singles.tile([128, BPC, NT], F32)
        nc.sync.dma_start(out=mkT, in_=mk_d.rearrange("p (b t) -> p b t", b=BPC))
        bpT = singles.tile([128, CB], F32)
        nc.sync.dma_start(out=bpT, in_=bp_d[:, :])

        # ---- the whole bulk-DMA program, emitted upfront: batch-0 quarters,
        # then weights interleaved ahead of the batch-1 quarters. Issue of
        # each transfer waits only on its pool buffer being free, so the DMA
        # engines read ahead as far as SBUF allows, decoupled from compute.
        wvT = singles.tile([128, CB, C], BF16)
        wpT = singles.tile([128, CB, C], BF16)
        for q in range(NQ):
            emit_quarter(0, q)
        nc.sync.dma_start(out=wvT, in_=wvt_d.rearrange("p (k c) -> p k c", k=CB))
        emit_quarter(1, 0)
        nc.sync.dma_start(out=wpT, in_=wpt_d.rearrange("p (k c) -> p k c", k=CB))
        for q in range(1, NQ):
            emit_quarter(1, q)

        ocb = singles.tile([128, BPC, CB], BF16)       # extracted out columns
        zTb = singles.tile([128, CB, BPC * H], BF16)   # packed z.T, both batches

        def emit_z(z_ps, l_ps, p_nat, tt, xin, b, t):
            # z += p.T @ x ; l += p.T @ ones  (whole-batch accumulation);
            # returned as three thunks the caller interleaves into the s-chain
            first, last = (t == 0), (t == NT - 1)
            def zcc(cc):
                return lambda: nc.tensor.matmul(
                    z_ps[:, cc * 512:(cc + 1) * 512],
                    p_nat[:, :],
                    xin[:, tt, cc * 512:(cc + 1) * 512],
                    start=first,
                    stop=last,
                )
            return (zcc(0), zcc(1), lambda: nc.tensor.matmul(
                l_ps, p_nat[:, :], ones_col, start=first, stop=last))

        zq = []
        epi_thunks = []
        for b in range(BPC):
            z_ps = psAcc.tile([H, C], F32, tag="acc")
            l_ps = psL.tile([H, 1], F32, tag="l")

            for t in range(NT):
                q, tt = divmod(t, TPQ)
                s_ps = psS.tile([128, H], F32, tag="s")
                p_nat = pp.tile([128, H], BF16, tag="p")
                xt = xt_tiles[(b, q)]
                xin = xin_tiles[(b, q)]

                # s_nat(128n, 16h) = sum_k xt_tile.T @ qhatT  (xt stationary, FWL)
                for k in range(CB):
                    nc.tensor.matmul(
                        s_ps,
                        xt[:, k, tt * 128:(tt + 1) * 128],
                        qhT[:, b, k, :],
                        start=(k == 0),
                        stop=(k == CB - 1),
                    )
                for f in (list(emit_z(*zq.pop(0))) if len(zq) >= 2 else []):
                    f()
                # p = exp(s + mask) with per-partition mask bias, straight from PSUM
                nc.scalar.activation(
                    out=p_nat,
                    in_=s_ps,
                    func=AF.Exp,
                    bias=mkT[:, b, t:t + 1],
                )
                zq.append((z_ps, l_ps, p_nat, tt, xin, b, t))
                if epi_thunks:
                    epi_thunks.pop(0)()

            # flush the pipelined z-chains for this batch's last n-tiles
            while zq:
                for f in emit_z(*zq.pop(0)):
                    f()

            # ---- epilogue thunks: z-finalize for this batch (interleaved
            # into the next batch's n-tile stream, one thunk per tile) ----
            def make_epilogue(b, z_ps, l_ps):
                th = []
                if b == 0:
                    # small matmul burst, slotted into the batch boundary by
                    # pool rotation (its buffer waits l_ps's reciprocal read):
                    # keeps the HAM clock gate open across the z-finalize dip
                    # so batch 1's first z-matmuls run at full clock
                    wb = psL.tile([1, 512], F32, tag="l", name="wb0")

                    def warmburst():
                        for i in range(3):
                            nc.tensor.matmul(wb, ones_col, wsrc,
                                             start=(i == 0), stop=(i == 2))
                    th.append(warmburst)
                linv = singles.tile([H, 1], F32, name=f"linv{b}")
                z_sb = singles.tile([H, C], F32, name=f"z_sb{b}")
                ztp = psT.tile([128, CB, H], F32, tag="tp", name=f"ztp{b}")

                th.append(lambda: nc.vector.reciprocal(out=linv, in_=l_ps))
                for hh in range(2):
                    th.append(lambda hh=hh: nc.vector.tensor_scalar_mul(
                        z_sb[:, hh * 512:(hh + 1) * 512],
                        z_ps[:, hh * 512:(hh + 1) * 512], linv))
                for k0 in range(0, CB, 2):
                    def tr(k0=k0):
                        for k in (k0, k0 + 1):
                            nc.tensor.transpose(
                                ztp[:, k, :], z_sb[:, k * 128:(k + 1) * 128],
                                ident[0:H, 0:H])
                    th.append(tr)
                th.append(lambda: nc.vector.tensor_copy(
                    out=zTb[:, :, b * H:(b + 1) * H],
                    in_=ztp.rearrange("p k h -> p k h")))
                return th

            epi_thunks.extend(make_epilogue(b, z_ps, l_ps))

        for th in epi_thunks:
            th()

        # ---- merged projections, both batches in one weight pass ----
        # out'T[c', (b,h)] = (z @ Wv.T).T via stationary Wv slices: output is
        # c-major so the block-diag extract is two strided DVE copies per batch
        OP = psT.tile([128, CB, BPC * H], F32, tag="tp", name="OP")
        for m in range(CB):
            for k in range(CB):
                nc.tensor.matmul(
                    OP[:, m, :],
                    wvT[:, k, m * 128:(m + 1) * 128],
                    zTb[:, k, :],
                    start=(k == 0),
                    stop=(k == CB - 1),
                )
        # ocb[p, b, j] = OP[p, j, b*H + 2j + (p >= 64)]
        for b in range(BPC):
            ev = OP[0:64, 0, b * H:b * H + 1]
            od = OP[64:128, 0, b * H + 1:b * H + 2]
            nc.vector.tensor_copy(
                out=ocb[0:64, b, :],
                in_=bass.AP(tensor=ev.tensor, offset=ev.offset,
                            ap=[ev.ap[0], [BPC * H + 2, CB]]))
            nc.vector.tensor_copy(
                out=ocb[64:128, b, :],
                in_=bass.AP(tensor=od.tensor, offset=od.offset,
                            ap=[od.ap[0], [BPC * H + 2, CB]]))

        # yT[c2, b] = (out @ Wp.T).T via stationary Wp slices
        YT = psL.tile([128, CB, BPC], F32, tag="l", name="YT")
        for m in range(CB):
            for j in range(CB):
                nc.tensor.matmul(
                    YT[:, m, :],
                    wpT[:, j, m * 128:(m + 1) * 128],
                    ocb[:, :, j],
                    start=(j == 0),
                    stop=(j == CB - 1),
                )
        y_sb = singles.tile([128, CB, BPC], F32)
        for b in range(BPC):
            nc.vector.tensor_tensor(
                out=y_sb[:, :, b], in0=YT[:, :, b], in1=bpT, op=ALU.add)
            nc.sync.dma_start(
                out=y_d[b, :].rearrange("(m p) -> p m", p=128), in_=y_sb[:, :, b]
            )

    nc.compile()
    return nc


def _ensure_ntff_hook():
    """The agent image's antenv lacks axon_hooks; synthesize it and install
    the ctypes NTFF profile hook from trn_boot so trace=True works."""
    import sys
    import types
    try:
        from antenv.axon_hooks import get_axon_ntff_profile_hook  # noqa: F401
        return
    except ImportError:
        pass
    import antenv
    mod = types.ModuleType("antenv.axon_hooks")
    state = {}
    mod.set_axon_ntff_profile_hook = lambda h: state.__setitem__("h", h)
    mod.get_axon_ntff_profile_hook = lambda: state.get("h")
    sys.modules["antenv.axon_hooks"] = mod
    antenv.axon_hooks = mod
    try:
        from trn_agent_boot.trn_boot import _ntff_profile_via_ctypes
        mod.set_axon_ntff_profile_hook(
            _ntff_profile_via_ctypes("/opt/axon/libaxon_pjrt.so")
        )
    except Exception:
        pass


_NC_CACHE = None


def _get_module():
    global _NC_CACHE
    if _NC_CACHE is None:
        _NC_CACHE = build_module()
    return _NC_CACHE


def _np_xt_dtype():
    import ml_dtypes
    return {BF16: ml_dtypes.bfloat16, FP8: ml_dtypes.float8_e4m3fn}[XT_DT]


def _prep_inputs(inputs):
    """Host-side prep: bf16/fp8 casts and per-partition-contiguous packing."""
    import ml_dtypes
    bf16 = ml_dtypes.bfloat16

    x = np.ascontiguousarray(inputs["x"], dtype=np.float32)       # (B,N,C)
    mask = np.ascontiguousarray(inputs["mask"], dtype=np.float32)
    Wq = np.asarray(inputs["Wq"], dtype=np.float32)
    Wk = np.asarray(inputs["Wk"], dtype=np.float32)

    # natural x, packed [b, q, p, (t c)]: partition p = n%128 within quarter
    xb = np.ascontiguousarray(
        x.reshape(B, NQ, TPQ, 128, C).transpose(0, 1, 3, 2, 4)
    ).reshape(B, NQ, 128, TPQ * C).astype(_np_xt_dtype())
    # transposed x, packed [b, q, p, (k n)]: partition p = c%128
    xtb = np.ascontiguousarray(
        x.transpose(0, 2, 1).reshape(B, CB, 128, NQ, NPQ).transpose(0, 3, 2, 1, 4)
    ).reshape(B, NQ, 128, CB * NPQ).astype(_np_xt_dtype())

    # qhat[b,h,:] = sum_d (x[b,0] @ Wq.T * scale)[h*64+d] * Wk[h*64+d,:]
    q = (x[:, 0, :].astype(np.float64) @ Wq.T.astype(np.float64)) * SCALE  # (B,C)
    qhd = q.reshape(B, H, D)
    Wkh = Wk.reshape(H, D, C).astype(np.float64)
    qhat = np.einsum("bhd,hdc->bhc", qhd, Wkh)                     # (B,H,C)
    qhT = qhat.transpose(0, 2, 1)                                  # (B,C,H)
    qhp = np.ascontiguousarray(
        qhT.reshape(NCORES, BPC, CB, 128, H).transpose(0, 3, 1, 2, 4)
    ).reshape(NCORES, 128, BPC * CB * H).astype(bf16)

    # mask_full packed per n-tile: (core, 128, BPC*NT)
    mask_full = np.concatenate(
        [np.zeros((B, 1), dtype=np.float32), mask], axis=1)        # (B,N)
    mkp = np.ascontiguousarray(
        mask_full.reshape(NCORES, BPC, NT, 128).transpose(0, 3, 1, 2)
    ).reshape(NCORES, 128, BPC * NT)

    def packw(w):
        wt = np.ascontiguousarray(np.asarray(w, dtype=np.float32).T)  # (C,C)
        return np.ascontiguousarray(
            wt.reshape(CB, 128, C).transpose(1, 0, 2)
        ).reshape(128, CB * C).astype(bf16)

    shared = {
        "WvT": packw(inputs["Wv"]),
        "WpT": packw(inputs["Wp"]),
        "bp": np.ascontiguousarray(
            np.asarray(inputs["bp"], dtype=np.float32).reshape(CB, 128).T),
    }
    in_maps = []
    for c in range(NCORES):
        sl = slice(c * BPC, (c + 1) * BPC)
        m = {
            "xb": xb[sl], "xtb": xtb[sl], "qhp": qhp[c], "mkp": mkp[c],
        }
        m.update(shared)
        in_maps.append(m)
    return in_maps


def run(inputs, trace=False):
    if trace:
        _ensure_ntff_hook()
    nc = _get_module()
    in_maps = _prep_inputs(inputs)
    res = bass_utils.run_bass_kernel_spmd(
        nc, in_maps, core_ids=list(range(NCORES)), trace=trace
    )
    ys = [res.results[c]["y"] for c in range(NCORES)]
    out = np.concatenate(ys, axis=0).reshape(B, 1, C)
    return out, res


def kernel(**inputs):
    out, _ = run(inputs, trace=False)
    return out


if __name__ == "__main__":
    rng = np.random.default_rng(0)
    ins = {
        "x": rng.standard_normal((B, N, C), dtype=np.float32),
        "mask": np.zeros((B, N - 1), dtype=np.float32),
        "Wq": (rng.standard_normal((C, C)) * 0.02).astype(np.float32),
        "Wk": (rng.standard_normal((C, C)) * 0.02).astype(np.float32),
        "Wv": (rng.standard_normal((C, C)) * 0.02).astype(np.float32),
        "Wp": (rng.standard_normal((C, C)) * 0.02).astype(np.float32),
        "bp": np.zeros((C,), dtype=np.float32),
    }
    y = kernel(**ins)
    print(y.shape, y.dtype, np.abs(y).mean())
